# revision 1
# baseline (speedup 1.0000x reference)
"""Real spherical harmonics Y_{l,m} (l_max=8) on 8 TRN2 NeuronCores.

Strategy: trivially data-parallel over the sample axis. Each core gets
250,112 samples (2M padded to 8*250,112). Per core, a Bass/Tile kernel
computes all 81 columns in fp16:
  - ACT: |phi|, z^2, st=sqrt(1-z^2), sin(phi), cos(phi)=Sin(pi/2-|phi|)
  - DVE/GPSIMD: (g_m, h_m) = st^m (cos m phi, sin m phi) via complex powers,
    scaled associated-Legendre recurrences A_{l,m} = C P~_{l,m}(z) written
    into concatenated per-chain tiles, then one broadcast tensor_tensor per
    (chain, side) produces output columns Y = A * g / A * h.
Output is stored column-major [81, S] fp16 per core; the host transposes,
reorders columns, casts to f32 and trims padding.
"""
import math
import sys

if "/opt/trn_rl_repo" not in sys.path:
    sys.path.insert(0, "/opt/trn_rl_repo")

import numpy as np

L = 8
NCOLS = (L + 1) ** 2  # 81
P = 128
BLOCKS = [652, 652, 650]  # per-core free-dim tile sizes (all even; sum*P = S_CORE)
S_CORE = P * sum(BLOCKS)  # 250112
N_CORES = 8
N_FULL = 2_000_000

# engine assignment knobs (tuned empirically)
CONFIG = {
    "squares_on_act": True,     # gh-chain g^2/h^2 on ScalarE instead of DVE
    "gps_sin_ms": (4, 5, 6, 7, 8),   # sin-side big output muls on GPSIMD for these m
    "gps_cos_ms": (),                # cos-side big output muls on GPSIMD
    "gps_prod_ms": (),               # gh product target (w3/w5/w7) index -> GPSIMD
}


def _dfact(n):
    r = 1
    while n > 1:
        r *= n
        n -= 2
    return r


def _consts():
    def K(l, m):
        return math.sqrt((2 * l + 1) / (4.0 * math.pi) * math.factorial(l - m) / math.factorial(l + m))

    SQ2 = math.sqrt(2.0)
    C = {}
    for m in range(0, L + 1):
        for l in range(m, L + 1):
            C[(l, m)] = (SQ2 if m > 0 else 1.0) * K(l, m)
    a = {}
    b = {}
    for m in range(0, L + 1):
        for l in range(m + 2, L + 1):
            a[(l, m)] = (2 * l - 1) / (l - m) * C[(l, m)] / C[(l - 1, m)]
            b[(l, m)] = -(l + m - 1) / (l - m) * C[(l, m)] / C[(l - 2, m)]
    seed_mm = {m: C[(m, m)] * _dfact(2 * m - 1) for m in range(0, L + 1)}
    seed_m1 = {m: C[(m + 1, m)] * _dfact(2 * m + 1) for m in range(0, L)}
    return C, a, b, seed_mm, seed_m1


def _row_order():
    """Our DRAM row order -> reference column index (l*l + l + m)."""
    rows = []
    for l in range(L + 1):
        rows.append((l, 0))
    for m in range(1, L + 1):
        for l in range(m, L + 1):
            rows.append((l, m))
        for l in range(m, L + 1):
            rows.append((l, -m))
    assert len(rows) == NCOLS
    return np.array([l * l + l + m for (l, m) in rows], dtype=np.int64)


def build_nc():
    from concourse import bacc, mybir, tile

    F32 = mybir.dt.float32
    F16 = mybir.dt.float16
    AF = mybir.ActivationFunctionType
    ALU = mybir.AluOpType

    _, a, b, seed_mm, seed_m1 = _consts()
    cfg = CONFIG

    nc = bacc.Bacc(None)
    ct_d = nc.dram_tensor("ct", [S_CORE], F32, kind="ExternalInput")
    ph_d = nc.dram_tensor("ph", [S_CORE], F32, kind="ExternalInput")
    out_d = nc.dram_tensor("out", [NCOLS, S_CORE], F16, kind="ExternalOutput")

    with tile.TileContext(nc) as tc:
        with (
            tc.tile_pool(name="io", bufs=2) as pio,
            tc.tile_pool(name="mid", bufs=2) as pmid,
            tc.tile_pool(name="gh", bufs=2) as pgh,
            tc.tile_pool(name="acat", bufs=2) as pacat,
            tc.tile_pool(name="outp", bufs=2) as pout,
            tc.tile_pool(name="cst", bufs=1) as pcst,
        ):
            halfpi = pcst.tile([P, 1], F32, tag="halfpi")
            nc.gpsimd.memset(halfpi[:], math.pi / 2)

            off = 0
            row0_m = {}  # first our-row of chain m's cos block
            r = L + 1
            for m in range(1, L + 1):
                row0_m[m] = r
                r += 2 * (L + 1 - m)

            for T in BLOCKS:
                span = P * T
                zf = pio.tile([P, T], F32, tag="zf")
                pf = pio.tile([P, T], F32, tag="pf")
                nc.sync.dma_start(out=zf[:], in_=ct_d[off:off + span].rearrange("(p t) -> p t", p=P))
                nc.sync.dma_start(out=pf[:], in_=ph_d[off:off + span].rearrange("(p t) -> p t", p=P))

                # ---- ACT prologue ----
                aph = pmid.tile([P, T], F32, tag="aph")
                nc.scalar.activation(aph[:], pf[:], AF.Abs)
                z2f = pmid.tile([P, T], F32, tag="z2f")
                nc.scalar.activation(z2f[:], zf[:], AF.Square)
                st = pmid.tile([P, T], F16, tag="st")
                nc.scalar.activation(st[:], z2f[:], AF.Sqrt, scale=-1.0, bias=1.0)
                sp = pmid.tile([P, T], F16, tag="sp")
                nc.scalar.activation(sp[:], pf[:], AF.Sin)
                cp = pmid.tile([P, T], F16, tag="cp")
                nc.scalar.activation(cp[:], aph[:], AF.Sin, scale=-1.0, bias=halfpi[:, :1])

                # fp16 casts of z and z^2
                z16 = pmid.tile([P, T], F16, tag="z16")
                nc.vector.tensor_copy(z16[:], zf[:])
                z216 = pmid.tile([P, T], F16, tag="z216")
                nc.vector.tensor_copy(z216[:], z2f[:])

                # ---- gh chain: w_m = (st e^{i phi})^m ----
                x = pgh.tile([P, T], F16, tag="g1")
                nc.vector.tensor_tensor(x[:], st[:], cp[:], ALU.mult)
                y = pgh.tile([P, T], F16, tag="h1")
                nc.vector.tensor_tensor(y[:], st[:], sp[:], ALU.mult)
                w = {1: (x, y)}

                def sq(i):
                    g_, h_ = w[i]
                    m2 = 2 * i
                    gt = pgh.tile([P, T], F16, tag=f"g{m2}")
                    ht = pgh.tile([P, T], F16, tag=f"h{m2}")
                    gA = pmid.tile([P, T], F16, tag="sqA")
                    hA = pmid.tile([P, T], F16, tag="sqB")
                    if cfg["squares_on_act"]:
                        nc.scalar.activation(gA[:], g_[:], AF.Square)
                        nc.scalar.activation(hA[:], h_[:], AF.Square)
                    else:
                        nc.vector.tensor_tensor(gA[:], g_[:], g_[:], ALU.mult)
                        nc.vector.tensor_tensor(hA[:], h_[:], h_[:], ALU.mult)
                    nc.vector.tensor_tensor(gt[:], gA[:], hA[:], ALU.subtract)
                    nc.vector.scalar_tensor_tensor(ht[:], g_[:], 2.0, h_[:], ALU.mult, ALU.mult)
                    w[m2] = (gt, ht)

                def prod(i, j):
                    (gi, hi), (gj, hj) = w[i], w[j]
                    m2 = i + j
                    eng = nc.gpsimd if m2 in cfg["gps_prod_ms"] else nc.vector
                    gt = pgh.tile([P, T], F16, tag=f"g{m2}")
                    ht = pgh.tile([P, T], F16, tag=f"h{m2}")
                    t1 = pmid.tile([P, T], F16, tag="p1")
                    t2 = pmid.tile([P, T], F16, tag="p2")
                    t3 = pmid.tile([P, T], F16, tag="p3")
                    t4 = pmid.tile([P, T], F16, tag="p4")
                    eng.tensor_tensor(t1[:], gi[:], gj[:], ALU.mult)
                    eng.tensor_tensor(t2[:], hi[:], hj[:], ALU.mult)
                    eng.tensor_tensor(gt[:], t1[:], t2[:], ALU.subtract)
                    eng.tensor_tensor(t3[:], gi[:], hj[:], ALU.mult)
                    eng.tensor_tensor(t4[:], hi[:], gj[:], ALU.mult)
                    eng.tensor_tensor(ht[:], t3[:], t4[:], ALU.add)
                    w[m2] = (gt, ht)

                sq(1); prod(2, 1); sq(2); prod(4, 1); sq(3); prod(6, 1); sq(4)

                # ---- m = 0 chain (columns are the A values directly) ----
                o0 = pout.tile([P, (L + 1) * T], F16, tag="o0")
                def s_(t_, d):  # slice d of a concatenated tile
                    return t_[:, d * T:(d + 1) * T]
                nc.gpsimd.memset(s_(o0, 0), seed_mm[0])
                nc.vector.tensor_scalar(s_(o0, 1), z16[:], seed_m1[0], None, ALU.mult)
                nc.vector.tensor_scalar(
                    s_(o0, 2), z216[:], a[(2, 0)] * seed_m1[0], b[(2, 0)] * seed_mm[0], ALU.mult, ALU.add
                )
                for l in range(3, L + 1):
                    u = pmid.tile([P, T], F16, tag="u")
                    nc.vector.scalar_tensor_tensor(u[:], s_(o0, l - 1), a[(l, 0)], z16[:], ALU.mult, ALU.mult)
                    nc.vector.scalar_tensor_tensor(s_(o0, l), s_(o0, l - 2), b[(l, 0)], u[:], ALU.mult, ALU.add)
                nc.sync.dma_start(
                    out=out_d[0:L + 1, off:off + span].rearrange("r (p t) -> p r t", p=P),
                    in_=o0.rearrange("p (r t) -> p r t", r=L + 1),
                )

                # ---- m >= 1 chains ----
                for m in range(1, L + 1):
                    k = L + 1 - m
                    acat = pacat.tile([P, k * T], F16, tag="acat")
                    nc.gpsimd.memset(s_(acat, 0), seed_mm[m])
                    if m + 1 <= L:
                        nc.vector.tensor_scalar(s_(acat, 1), z16[:], seed_m1[m], None, ALU.mult)
                    if m + 2 <= L:
                        nc.vector.tensor_scalar(
                            s_(acat, 2), z216[:],
                            a[(m + 2, m)] * seed_m1[m], b[(m + 2, m)] * seed_mm[m],
                            ALU.mult, ALU.add,
                        )
                    for l in range(m + 3, L + 1):
                        d = l - m
                        u = pmid.tile([P, T], F16, tag="u")
                        nc.vector.scalar_tensor_tensor(u[:], s_(acat, d - 1), a[(l, m)], z16[:], ALU.mult, ALU.mult)
                        nc.vector.scalar_tensor_tensor(s_(acat, d), s_(acat, d - 2), b[(l, m)], u[:], ALU.mult, ALU.add)

                    gm, hm = w[m]
                    a3 = acat.rearrange("p (r t) -> p r t", r=k)
                    for side, trig, gps_set in (("c", gm, cfg["gps_cos_ms"]), ("s", hm, cfg["gps_sin_ms"])):
                        ot = pout.tile([P, k * T], F16, tag=f"o{side}")
                        eng = nc.gpsimd if m in gps_set else nc.vector
                        eng.tensor_tensor(
                            ot.rearrange("p (r t) -> p r t", r=k),
                            a3,
                            trig[:, None, :].broadcast_to((P, k, T)),
                            ALU.mult,
                        )
                        r0 = row0_m[m] + (k if side == "s" else 0)
                        nc.sync.dma_start(
                            out=out_d[r0:r0 + k, off:off + span].rearrange("r (p t) -> p r t", p=P),
                            in_=ot.rearrange("p (r t) -> p r t", r=k),
                        )
                off += span

    nc.finalize()
    return nc


_NC_CACHE = {}


def get_nc():
    if "nc" not in _NC_CACHE:
        _NC_CACHE["nc"] = build_nc()
    return _NC_CACHE["nc"]


def _numpy_fallback(l_max, ct, ph):
    ct = ct.astype(np.float64)
    ph = ph.astype(np.float64)
    st = np.sqrt(np.maximum(1.0 - ct * ct, 0.0))
    Pd = {(0, 0): np.ones_like(ct)}
    for m in range(1, l_max + 1):
        Pd[(m, m)] = Pd[(m - 1, m - 1)] * st * (2 * m - 1)
    for m in range(0, l_max):
        Pd[(m + 1, m)] = ct * (2 * m + 1) * Pd[(m, m)]
    for m in range(0, l_max + 1):
        for l in range(m + 2, l_max + 1):
            Pd[(l, m)] = ((2 * l - 1) * ct * Pd[(l - 1, m)] - (l + m - 1) * Pd[(l - 2, m)]) / (l - m)
    cols = []
    sq2 = math.sqrt(2.0)
    for l in range(l_max + 1):
        for m in range(-l, l + 1):
            am = abs(m)
            k = math.sqrt((2 * l + 1) / (4.0 * math.pi) * math.factorial(l - am) / math.factorial(l + am))
            if m < 0:
                cols.append((sq2 * k) * Pd[(l, am)] * np.sin(am * ph))
            elif m == 0:
                cols.append(k * Pd[(l, 0)])
            else:
                cols.append((sq2 * k) * Pd[(l, m)] * np.cos(m * ph))
    return np.stack(cols, axis=1).astype(np.float32)


def make_in_maps(ct, ph):
    n = ct.shape[0]
    ctp = np.zeros(N_CORES * S_CORE, np.float32)
    php = np.zeros(N_CORES * S_CORE, np.float32)
    ctp[:n] = ct
    php[:n] = ph
    return [
        {"ct": ctp[i * S_CORE:(i + 1) * S_CORE], "ph": php[i * S_CORE:(i + 1) * S_CORE]}
        for i in range(N_CORES)
    ]


def assemble(results, n):
    """results: list of per-core dicts with 'out' [81, S_CORE] fp16."""
    rows = np.concatenate([np.asarray(r["out"]) for r in results], axis=1)[:, :n]
    out = np.empty((n, NCOLS), dtype=np.float32)
    out[:, _row_order()] = rows.T.astype(np.float32)
    return out


def kernel(l_max, cos_theta, phi):
    l_max = int(np.asarray(l_max))
    ct = np.asarray(cos_theta, dtype=np.float32).ravel()
    ph = np.asarray(phi, dtype=np.float32).ravel()
    if l_max != L or ct.shape[0] != N_FULL:
        return _numpy_fallback(l_max, ct, ph)

    from concourse.bass_utils import run_bass_kernel_spmd

    nc = get_nc()
    in_maps = make_in_maps(ct, ph)
    res = run_bass_kernel_spmd(nc, in_maps, core_ids=list(range(N_CORES)), trace=False)
    return assemble(res.results, N_FULL)


if __name__ == "__main__":
    rng = np.random.default_rng(7)
    n = N_FULL
    ct = rng.uniform(-0.999, 0.999, n).astype(np.float32)
    ph = rng.uniform(-math.pi, math.pi, n).astype(np.float32)
    got = kernel(np.int64(L), ct, ph)
    exp = _numpy_fallback(L, ct, ph)
    rel = np.linalg.norm(got - exp) / np.linalg.norm(exp)
    print("rel err vs numpy ref:", rel)


# revision 3
# speedup vs baseline: 157.8868x; 157.8868x over previous
"""Real spherical harmonics Y_{l,m} (l_max=8) on 8 TRN2 NeuronCores.

Strategy: trivially data-parallel over the sample axis. Each core gets
250,112 samples (2M padded to 8*250,112). Per core, a Bass/Tile kernel
computes all 81 columns in fp16:
  - ACT: |phi|, z^2, st=sqrt(1-z^2), sin(phi), cos(phi)=Sin(pi/2-|phi|)
  - DVE/GPSIMD: (g_m, h_m) = st^m (cos m phi, sin m phi) via complex powers,
    scaled associated-Legendre recurrences A_{l,m} = C P~_{l,m}(z) written
    into concatenated per-chain tiles, then one broadcast tensor_tensor per
    (chain, side) produces output columns Y = A * g / A * h.
Output is stored column-major [81, S] fp16 per core; the host transposes,
reorders columns, casts to f32 and trims padding.
"""
import math
import sys

if "/opt/trn_rl_repo" not in sys.path:
    sys.path.insert(0, "/opt/trn_rl_repo")

import numpy as np

L = 8
NCOLS = (L + 1) ** 2  # 81
P = 128
BLOCKS = [652, 652, 650]  # per-core free-dim tile sizes (all even; sum*P = S_CORE)
S_CORE = P * sum(BLOCKS)  # 250112
N_CORES = 8
N_FULL = 2_000_000

# engine assignment knobs (tuned empirically)
CONFIG = {
    "squares_on_act": True,     # gh-chain g^2/h^2 on ScalarE instead of DVE
    "gps_sin_ms": (4, 5, 6, 7, 8),   # sin-side big output muls on GPSIMD for these m
    "gps_cos_ms": (),                # cos-side big output muls on GPSIMD
    "gps_prod_ms": (),               # gh product target (w3/w5/w7) index -> GPSIMD
}


def _dfact(n):
    r = 1
    while n > 1:
        r *= n
        n -= 2
    return r


def _consts():
    def K(l, m):
        return math.sqrt((2 * l + 1) / (4.0 * math.pi) * math.factorial(l - m) / math.factorial(l + m))

    SQ2 = math.sqrt(2.0)
    C = {}
    for m in range(0, L + 1):
        for l in range(m, L + 1):
            C[(l, m)] = (SQ2 if m > 0 else 1.0) * K(l, m)
    a = {}
    b = {}
    for m in range(0, L + 1):
        for l in range(m + 2, L + 1):
            a[(l, m)] = (2 * l - 1) / (l - m) * C[(l, m)] / C[(l - 1, m)]
            b[(l, m)] = -(l + m - 1) / (l - m) * C[(l, m)] / C[(l - 2, m)]
    seed_mm = {m: C[(m, m)] * _dfact(2 * m - 1) for m in range(0, L + 1)}
    seed_m1 = {m: C[(m + 1, m)] * _dfact(2 * m + 1) for m in range(0, L)}
    return C, a, b, seed_mm, seed_m1


def _row_order():
    """Our DRAM row order -> reference column index (l*l + l + m)."""
    rows = []
    for l in range(L + 1):
        rows.append((l, 0))
    for m in range(1, L + 1):
        for l in range(m, L + 1):
            rows.append((l, m))
        for l in range(m, L + 1):
            rows.append((l, -m))
    assert len(rows) == NCOLS
    return np.array([l * l + l + m for (l, m) in rows], dtype=np.int64)


def build_nc(repeat=1):
    from concourse import bacc, mybir, tile

    F32 = mybir.dt.float32
    F16 = mybir.dt.float16
    AF = mybir.ActivationFunctionType
    ALU = mybir.AluOpType

    _, a, b, seed_mm, seed_m1 = _consts()
    cfg = CONFIG

    nc = bacc.Bacc(None)
    ct_d = nc.dram_tensor("ct", [S_CORE], F32, kind="ExternalInput")
    ph_d = nc.dram_tensor("ph", [S_CORE], F32, kind="ExternalInput")
    out_d = nc.dram_tensor("out", [NCOLS, S_CORE], F16, kind="ExternalOutput")

    with tile.TileContext(nc) as tc:
        with (
            tc.tile_pool(name="io", bufs=2) as pio,
            tc.tile_pool(name="mid", bufs=2) as pmid,
            tc.tile_pool(name="gh", bufs=2) as pgh,
            tc.tile_pool(name="acat", bufs=2) as pacat,
            tc.tile_pool(name="outp", bufs=2) as pout,
            tc.tile_pool(name="cst", bufs=1) as pcst,
        ):
            halfpi = pcst.tile([P, 1], F32, tag="halfpi")
            nc.gpsimd.memset(halfpi[:], math.pi / 2)

            off = 0
            row0_m = {}  # first our-row of chain m's cos block
            r = L + 1
            for m in range(1, L + 1):
                row0_m[m] = r
                r += 2 * (L + 1 - m)

            for T in BLOCKS * repeat:
                if off >= S_CORE:
                    off = 0  # timing amplification: redo the same work
                span = P * T
                zf = pio.tile([P, T], F32, tag="zf")
                pf = pio.tile([P, T], F32, tag="pf")
                nc.sync.dma_start(out=zf[:], in_=ct_d[off:off + span].rearrange("(p t) -> p t", p=P))
                nc.sync.dma_start(out=pf[:], in_=ph_d[off:off + span].rearrange("(p t) -> p t", p=P))

                # ---- ACT prologue ----
                aph = pmid.tile([P, T], F32, tag="aph")
                nc.scalar.activation(aph[:], pf[:], AF.Abs)
                z2f = pmid.tile([P, T], F32, tag="z2f")
                nc.scalar.activation(z2f[:], zf[:], AF.Square)
                st = pmid.tile([P, T], F16, tag="st")
                nc.scalar.activation(st[:], z2f[:], AF.Sqrt, scale=-1.0, bias=1.0)
                sp = pmid.tile([P, T], F16, tag="sp")
                nc.scalar.activation(sp[:], pf[:], AF.Sin)
                cp = pmid.tile([P, T], F16, tag="cp")
                nc.scalar.activation(cp[:], aph[:], AF.Sin, scale=-1.0, bias=halfpi[:, :1])

                # fp16 casts of z and z^2
                z16 = pmid.tile([P, T], F16, tag="z16")
                nc.vector.tensor_copy(z16[:], zf[:])
                z216 = pmid.tile([P, T], F16, tag="z216")
                nc.vector.tensor_copy(z216[:], z2f[:])

                # ---- gh chain: w_m = (st e^{i phi})^m ----
                x = pgh.tile([P, T], F16, tag="g1")
                nc.vector.tensor_tensor(x[:], st[:], cp[:], ALU.mult)
                y = pgh.tile([P, T], F16, tag="h1")
                nc.vector.tensor_tensor(y[:], st[:], sp[:], ALU.mult)
                w = {1: (x, y)}

                def sq(i):
                    g_, h_ = w[i]
                    m2 = 2 * i
                    gt = pgh.tile([P, T], F16, tag=f"g{m2}")
                    ht = pgh.tile([P, T], F16, tag=f"h{m2}")
                    gA = pmid.tile([P, T], F16, tag="sqA")
                    hA = pmid.tile([P, T], F16, tag="sqB")
                    if cfg["squares_on_act"]:
                        nc.scalar.activation(gA[:], g_[:], AF.Square)
                        nc.scalar.activation(hA[:], h_[:], AF.Square)
                    else:
                        nc.vector.tensor_tensor(gA[:], g_[:], g_[:], ALU.mult)
                        nc.vector.tensor_tensor(hA[:], h_[:], h_[:], ALU.mult)
                    nc.vector.tensor_tensor(gt[:], gA[:], hA[:], ALU.subtract)
                    nc.vector.scalar_tensor_tensor(ht[:], g_[:], 2.0, h_[:], ALU.mult, ALU.mult)
                    w[m2] = (gt, ht)

                def prod(i, j):
                    (gi, hi), (gj, hj) = w[i], w[j]
                    m2 = i + j
                    eng = nc.gpsimd if m2 in cfg["gps_prod_ms"] else nc.vector
                    gt = pgh.tile([P, T], F16, tag=f"g{m2}")
                    ht = pgh.tile([P, T], F16, tag=f"h{m2}")
                    t1 = pmid.tile([P, T], F16, tag="p1")
                    t2 = pmid.tile([P, T], F16, tag="p2")
                    t3 = pmid.tile([P, T], F16, tag="p3")
                    t4 = pmid.tile([P, T], F16, tag="p4")
                    eng.tensor_tensor(t1[:], gi[:], gj[:], ALU.mult)
                    eng.tensor_tensor(t2[:], hi[:], hj[:], ALU.mult)
                    eng.tensor_tensor(gt[:], t1[:], t2[:], ALU.subtract)
                    eng.tensor_tensor(t3[:], gi[:], hj[:], ALU.mult)
                    eng.tensor_tensor(t4[:], hi[:], gj[:], ALU.mult)
                    eng.tensor_tensor(ht[:], t3[:], t4[:], ALU.add)
                    w[m2] = (gt, ht)

                sq(1); prod(2, 1); sq(2); prod(4, 1); sq(3); prod(6, 1); sq(4)

                # ---- m = 0 chain (columns are the A values directly) ----
                o0 = pout.tile([P, (L + 1) * T], F16, tag="o0")
                def s_(t_, d):  # slice d of a concatenated tile
                    return t_[:, d * T:(d + 1) * T]
                nc.gpsimd.memset(s_(o0, 0), seed_mm[0])
                nc.vector.tensor_scalar(s_(o0, 1), z16[:], seed_m1[0], None, ALU.mult)
                nc.vector.tensor_scalar(
                    s_(o0, 2), z216[:], a[(2, 0)] * seed_m1[0], b[(2, 0)] * seed_mm[0], ALU.mult, ALU.add
                )
                for l in range(3, L + 1):
                    u = pmid.tile([P, T], F16, tag="u")
                    nc.vector.scalar_tensor_tensor(u[:], s_(o0, l - 1), a[(l, 0)], z16[:], ALU.mult, ALU.mult)
                    nc.vector.scalar_tensor_tensor(s_(o0, l), s_(o0, l - 2), b[(l, 0)], u[:], ALU.mult, ALU.add)
                nc.sync.dma_start(
                    out=out_d[0:L + 1, off:off + span].rearrange("r (p t) -> p r t", p=P),
                    in_=o0.rearrange("p (r t) -> p r t", r=L + 1),
                )

                # ---- m >= 1 chains ----
                for m in range(1, L + 1):
                    k = L + 1 - m
                    acat = pacat.tile([P, k * T], F16, tag="acat")
                    nc.gpsimd.memset(s_(acat, 0), seed_mm[m])
                    if m + 1 <= L:
                        nc.vector.tensor_scalar(s_(acat, 1), z16[:], seed_m1[m], None, ALU.mult)
                    if m + 2 <= L:
                        nc.vector.tensor_scalar(
                            s_(acat, 2), z216[:],
                            a[(m + 2, m)] * seed_m1[m], b[(m + 2, m)] * seed_mm[m],
                            ALU.mult, ALU.add,
                        )
                    for l in range(m + 3, L + 1):
                        d = l - m
                        u = pmid.tile([P, T], F16, tag="u")
                        nc.vector.scalar_tensor_tensor(u[:], s_(acat, d - 1), a[(l, m)], z16[:], ALU.mult, ALU.mult)
                        nc.vector.scalar_tensor_tensor(s_(acat, d), s_(acat, d - 2), b[(l, m)], u[:], ALU.mult, ALU.add)

                    gm, hm = w[m]
                    a3 = acat.rearrange("p (r t) -> p r t", r=k)
                    for side, trig, gps_set in (("c", gm, cfg["gps_cos_ms"]), ("s", hm, cfg["gps_sin_ms"])):
                        ot = pout.tile([P, k * T], F16, tag=f"o{side}")
                        eng = nc.gpsimd if m in gps_set else nc.vector
                        eng.tensor_tensor(
                            ot.rearrange("p (r t) -> p r t", r=k),
                            a3,
                            trig[:, None, :].broadcast_to((P, k, T)),
                            ALU.mult,
                        )
                        r0 = row0_m[m] + (k if side == "s" else 0)
                        nc.sync.dma_start(
                            out=out_d[r0:r0 + k, off:off + span].rearrange("r (p t) -> p r t", p=P),
                            in_=ot.rearrange("p (r t) -> p r t", r=k),
                        )
                off += span

    nc.finalize()
    return nc


_NC_CACHE = {}


def get_nc():
    if "nc" not in _NC_CACHE:
        _NC_CACHE["nc"] = build_nc()
    return _NC_CACHE["nc"]


def _numpy_fallback(l_max, ct, ph):
    ct = ct.astype(np.float64)
    ph = ph.astype(np.float64)
    st = np.sqrt(np.maximum(1.0 - ct * ct, 0.0))
    Pd = {(0, 0): np.ones_like(ct)}
    for m in range(1, l_max + 1):
        Pd[(m, m)] = Pd[(m - 1, m - 1)] * st * (2 * m - 1)
    for m in range(0, l_max):
        Pd[(m + 1, m)] = ct * (2 * m + 1) * Pd[(m, m)]
    for m in range(0, l_max + 1):
        for l in range(m + 2, l_max + 1):
            Pd[(l, m)] = ((2 * l - 1) * ct * Pd[(l - 1, m)] - (l + m - 1) * Pd[(l - 2, m)]) / (l - m)
    cols = []
    sq2 = math.sqrt(2.0)
    for l in range(l_max + 1):
        for m in range(-l, l + 1):
            am = abs(m)
            k = math.sqrt((2 * l + 1) / (4.0 * math.pi) * math.factorial(l - am) / math.factorial(l + am))
            if m < 0:
                cols.append((sq2 * k) * Pd[(l, am)] * np.sin(am * ph))
            elif m == 0:
                cols.append(k * Pd[(l, 0)])
            else:
                cols.append((sq2 * k) * Pd[(l, m)] * np.cos(m * ph))
    return np.stack(cols, axis=1).astype(np.float32)


def make_in_maps(ct, ph):
    n = ct.shape[0]
    ctp = np.zeros(N_CORES * S_CORE, np.float32)
    php = np.zeros(N_CORES * S_CORE, np.float32)
    ctp[:n] = ct
    php[:n] = ph
    return [
        {"ct": ctp[i * S_CORE:(i + 1) * S_CORE], "ph": php[i * S_CORE:(i + 1) * S_CORE]}
        for i in range(N_CORES)
    ]


def assemble(results, n):
    """results: list of per-core dicts with 'out' [81, S_CORE] fp16."""
    rows = np.concatenate([np.asarray(r["out"]) for r in results], axis=1)[:, :n]
    out = np.empty((n, NCOLS), dtype=np.float32)
    out[:, _row_order()] = rows.T.astype(np.float32)
    return out


def kernel(l_max, cos_theta, phi):
    l_max = int(np.asarray(l_max))
    ct = np.asarray(cos_theta, dtype=np.float32).ravel()
    ph = np.asarray(phi, dtype=np.float32).ravel()
    if l_max != L or ct.shape[0] != N_FULL:
        return _numpy_fallback(l_max, ct, ph)

    from concourse.bass_utils import run_bass_kernel_spmd

    nc = get_nc()
    in_maps = make_in_maps(ct, ph)
    res = run_bass_kernel_spmd(nc, in_maps, core_ids=list(range(N_CORES)), trace=False)
    return assemble(res.results, N_FULL)


if __name__ == "__main__":
    rng = np.random.default_rng(7)
    n = N_FULL
    ct = rng.uniform(-0.999, 0.999, n).astype(np.float32)
    ph = rng.uniform(-math.pi, math.pi, n).astype(np.float32)
    got = kernel(np.int64(L), ct, ph)
    exp = _numpy_fallback(L, ct, ph)
    rel = np.linalg.norm(got - exp) / np.linalg.norm(exp)
    print("rel err vs numpy ref:", rel)
